# revision 2
# baseline (speedup 1.0000x reference)
"""Multi-head attention (B=2, S=2048, D=1024, H=16) on 8 TRN2 NeuronCores.

Sharding: data-parallel over batch (cores 0-3 -> batch 0, cores 4-7 -> batch 1)
and tensor-parallel over heads (4 heads per core, via column shards of
wq/wk/wv and row shards of wo). Each core computes a partial output
projection [S, D]; the host sums the 4 partials per batch (the "all-reduce"
after the output projection).

Per-core pipeline (all matmuls in fp32r: full PE speed, ~1.5e-4 rel err):
  x -> PE-transpose -> xT;  QT/KT/VT head-pair projections;  V via PE
  transpose with appended ones columns;  scores kept transposed as tiles
  [128 k x 1024 q];  fully-masked causal tiles skipped, diagonal strips
  masked multiplicatively post-exp;  softmax without max-subtraction (exp
  row sums come free from the ones column of V);  O^T accumulated in PSUM;
  normalization via reciprocal + DMA broadcast;  partial out = O^T.T @ wo.
"""

import numpy as np

import concourse.bass as bass
import concourse.tile as tile
from concourse import bacc, mybir
from concourse.bass_utils import run_bass_kernel_spmd
from concourse.masks import make_identity, make_upper_triangular

B, S, D, H = 2, 2048, 1024, 16
DK = D // H          # 64
N_CORES = 8
HEADS_PER_CORE = H // 4      # 4
NPAIR = HEADS_PER_CORE // 2  # 2 head pairs per core
ND = D // 128        # 8 d-tiles
NS = S // 128        # 16 s-tiles / k-tiles
QCH = 1024           # q processed in chunks of 1024 (2 x 512 matmuls)
NQC = S // QCH       # 2 q-chunks

f32 = mybir.dt.float32
f32r = mybir.dt.float32r

_MODE_NONE, _MODE_CAUSAL, _MODE_GENERIC = 0, 1, 2


def _build_nc(mask_mode: int):
    nc = bacc.Bacc("TRN2", target_bir_lowering=False, debug=False,
                   num_devices=N_CORES)
    x_d = nc.dram_tensor("x", [S, D], f32, kind="ExternalInput").ap()
    wq_d = nc.dram_tensor("wq", [D, 256], f32, kind="ExternalInput").ap()
    wk_d = nc.dram_tensor("wk", [D, 256], f32, kind="ExternalInput").ap()
    wv_d = nc.dram_tensor("wv", [D, 256], f32, kind="ExternalInput").ap()
    wo_d = nc.dram_tensor("wo", [256, D], f32, kind="ExternalInput").ap()
    if mask_mode == _MODE_GENERIC:
        # multiplicative mask, already transposed: mexp[k, q] = exp(-1e9*mask[q, k])
        mexp_d = nc.dram_tensor("mexp", [S, S], f32, kind="ExternalInput").ap()
    out_d = nc.dram_tensor("out", [S, D], f32, kind="ExternalOutput").ap()
    sums_d = nc.dram_tensor("sums_scratch", [HEADS_PER_CORE * S], f32).ap()
    rcp_d = nc.dram_tensor("rcp_scratch", [HEADS_PER_CORE * S], f32).ap()

    with tile.TileContext(nc) as tc:
        with (
            tc.tile_pool(name="const", bufs=1) as const,
            tc.tile_pool(name="big", bufs=1) as big,
            tc.tile_pool(name="pair", bufs=1) as pairp,
            tc.tile_pool(name="vext", bufs=1) as vextp,
            tc.tile_pool(name="ptp", bufs=3) as ptp,
            tc.tile_pool(name="work", bufs=3) as work,
            tc.tile_pool(name="norm", bufs=1) as normp,
            tc.tile_pool(name="psA", bufs=2, space="PSUM") as psA,
            tc.tile_pool(name="psOT", bufs=2, space="PSUM") as psOT,
        ):
            def cp(use_scalar, out, in_):
                if use_scalar:
                    nc.scalar.copy(out, in_)
                else:
                    nc.vector.tensor_copy(out, in_)

            # ---------------- constants
            ident = const.tile([128, 128], f32)
            make_identity(nc, ident[:])
            ident_r = const.tile([128, 128], f32r)
            nc.vector.tensor_copy(ident_r[:], ident[:])
            tri_f = work.tile([128, 128], f32, name="tri_f", tag="xstage")
            make_upper_triangular(nc, tri_f[:], val=1.0, diag=True)
            tri_r = const.tile([128, 128], f32r)
            nc.vector.tensor_copy(tri_r[:], tri_f[:])
            ones_col = const.tile([128, 1], f32)
            nc.vector.memset(ones_col[:], 1.0)

            # ---------------- weights: load + round to fp32r
            w_r = {}
            for name, dram in (("wq", wq_d), ("wk", wk_d), ("wv", wv_d)):
                wr = big.tile([128, ND, 256], f32r, name=f"{name}_r",
                              tag=f"{name}_r")
                for dt_i in range(ND):
                    wf = work.tile([128, 256], f32, name="wf", tag="xstage")
                    nc.sync.dma_start(wf[:], dram[dt_i * 128:(dt_i + 1) * 128, :])
                    nc.vector.tensor_copy(wr[:, dt_i, :], wf[:])
                w_r[name] = wr
            wo_r = big.tile([128, 2, D], f32r)
            for t2 in range(2):
                wf2 = work.tile([128, D], f32, name="wf2", tag="xstage")
                nc.sync.dma_start(wf2[:], wo_d[t2 * 128:(t2 + 1) * 128, :])
                nc.vector.tensor_copy(wo_r[:, t2, :], wf2[:])

            # ---------------- x load + transpose -> xT fp32r [128, ND, S]
            xT = big.tile([128, ND, S], f32r)
            for st in range(NS):
                x_t = work.tile([128, D], f32, name="x_t", tag="xstage")
                nc.sync.dma_start(x_t[:], x_d[st * 128:(st + 1) * 128, :])
                tp = psA.tile([128, 1024], f32, name="tp", tag="psA")
                for dt_i in range(ND):
                    nc.tensor.transpose(
                        tp[:, dt_i * 128:(dt_i + 1) * 128],
                        x_t[:, dt_i * 128:(dt_i + 1) * 128], ident[:])
                cp(st % 2 == 0, xT[:, :, st * 128:(st + 1) * 128],
                   tp[:].rearrange("p (d s) -> p d s", d=ND))

            ot_all = big.tile([128, NPAIR, S], f32r)  # normalized O^T, head pairs

            for pair in range(NPAIR):
                # ------------ projections for this head pair (2 heads packed)
                co = pair * 128
                proj = {}
                for name in ("wq", "wk", "wv"):
                    pr = pairp.tile([128, S], f32r, name=f"proj_{name}",
                                    tag=f"proj_{name}")
                    for qc4 in range(4):   # 4 chunks of 512
                        pj = psA.tile([128, 1024], f32, name="pj", tag="psA")
                        for dt_i in range(ND):
                            nc.tensor.matmul(
                                pj[:, 0:512],
                                w_r[name][:, dt_i, co:co + 128],
                                xT[:, dt_i, qc4 * 512:(qc4 + 1) * 512],
                                start=(dt_i == 0), stop=(dt_i == ND - 1))
                        cp(qc4 % 2 == 0, pr[:, qc4 * 512:(qc4 + 1) * 512],
                           pj[:, 0:512])
                    proj[name] = pr

                # ------------ V_ext: [VA(0:64)|onesA(64)|VB(65:129)|onesB(129)]
                v_ext = vextp.tile([128, NS, 130], f32r, name="v_ext",
                                   tag="v_ext")
                for kt in range(NS):
                    vp = psA.tile([128, 1024], f32r, name="vp", tag="psA")
                    nc.tensor.transpose(vp[:, 0:128],
                                        proj["wv"][:, kt * 128:(kt + 1) * 128],
                                        ident_r[:])
                    cp(kt % 2 == 0, v_ext[:, kt, 0:64], vp[:, 0:64])
                    cp(kt % 2 == 1, v_ext[:, kt, 65:129], vp[:, 64:128])
                    nc.vector.tensor_copy(v_ext[:, kt, 64:65], ones_col[:])
                    nc.vector.tensor_copy(v_ext[:, kt, 129:130], ones_col[:])

                for hh in range(2):      # head within pair
                    head = pair * 2 + hh
                    qt_h = proj["wq"][hh * 64:(hh + 1) * 64, :]
                    kt_h = proj["wk"][hh * 64:(hh + 1) * 64, :]
                    vx_off = hh * 65
                    for qp in range(NQC):    # q-chunks of 1024
                        ot = psOT.tile([65, QCH], f32, name="ot", tag="ot")
                        kt_end = 8 * (qp + 1) if mask_mode == _MODE_CAUSAL else NS
                        for kt in range(kt_end):
                            lo = (max(0, kt * 128 - qp * QCH)
                                  if mask_mode == _MODE_CAUSAL else 0)
                            st_ps = psA.tile([128, 1024], f32, name="st_ps",
                                             tag="psA")
                            for sub in range(2):
                                a, b = max(lo, sub * 512), (sub + 1) * 512
                                if a >= b:
                                    continue
                                nc.tensor.matmul(
                                    st_ps[:, a:b],
                                    kt_h[:, kt * 128:(kt + 1) * 128],
                                    qt_h[:, qp * QCH + a:qp * QCH + b],
                                    start=True, stop=True)
                            pt = ptp.tile([128, QCH], f32r, name="pt", tag="pt")
                            nc.scalar.activation(
                                pt[:, lo:QCH], st_ps[:, lo:QCH],
                                mybir.ActivationFunctionType.Exp,
                                scale=1.0 / float(np.sqrt(DK)))
                            if mask_mode == _MODE_CAUSAL and kt * 128 >= qp * QCH:
                                nc.vector.tensor_mul(pt[:, lo:lo + 128],
                                                     pt[:, lo:lo + 128],
                                                     tri_r[:])
                            elif mask_mode == _MODE_GENERIC:
                                mm_t = work.tile([128, QCH], f32, name="mm_t",
                                                 tag="mm_t")
                                nc.sync.dma_start(
                                    mm_t[:],
                                    mexp_d[kt * 128:(kt + 1) * 128,
                                           qp * QCH:(qp + 1) * QCH])
                                nc.vector.tensor_mul(pt[:], pt[:], mm_t[:])
                            for sub in range(2):
                                a, b = max(lo, sub * 512), (sub + 1) * 512
                                if a >= b:
                                    continue
                                nc.tensor.matmul(
                                    ot[:, a:b],
                                    v_ext[:, kt, vx_off:vx_off + 65],
                                    pt[:, a:b],
                                    start=(kt == 0), stop=(kt == kt_end - 1))
                        # ------ normalize: recip of sums row, broadcast, mul
                        off = head * S + qp * QCH
                        sums_sb = normp.tile([1, QCH], f32, name="sums_sb",
                                             tag="sums_sb")
                        cp(qp % 2 == 0, sums_sb[:], ot[64:65, :])
                        nc.sync.dma_start(sums_d[None, off:off + QCH],
                                          sums_sb[0:1, :])
                        rsr = normp.tile([128, 16], f32, name="rsr", tag="rsr")
                        nc.sync.dma_start(
                            rsr[:, 0:8], sums_d[off:off + QCH].rearrange(
                                "(p f) -> p f", f=8))
                        nc.vector.reciprocal(out=rsr[:, 8:16], in_=rsr[:, 0:8])
                        nc.sync.dma_start(
                            rcp_d[off:off + QCH].rearrange(
                                "(p f) -> p f", f=8), rsr[:, 8:16])
                        rb = normp.tile([64, QCH], f32, name="rb", tag="rb")
                        nc.sync.dma_start(
                            rb[:], rcp_d[None, off:off + QCH].to_broadcast(
                                (64, QCH)))
                        nc.vector.tensor_mul(
                            ot_all[hh * 64:(hh + 1) * 64, pair,
                                   qp * QCH:(qp + 1) * QCH],
                            ot[0:64, :], rb[:])

            # ---------------- output projection (partial): out = OT_all.T @ wo
            for qt in range(NS):
                op = psA.tile([128, 1024], f32, name="op", tag="psA")
                for half in range(2):
                    for pair in range(NPAIR):
                        nc.tensor.matmul(
                            op[:, half * 512:(half + 1) * 512],
                            ot_all[:, pair, qt * 128:(qt + 1) * 128],
                            wo_r[:, pair, half * 512:(half + 1) * 512],
                            start=(pair == 0), stop=(pair == NPAIR - 1))
                o_sb = work.tile([128, D], f32, name="o_sb", tag="xstage")
                cp(qt % 2 == 0, o_sb[:], op[:])
                nc.sync.dma_start(out_d[qt * 128:(qt + 1) * 128, :], o_sb[:])

    nc.compile()
    return nc


_NC_CACHE = {}


def kernel(query_input, mask, wq, wk, wv, wo, _profile=False):
    query_input = np.asarray(query_input, dtype=np.float32)
    mask2d = np.asarray(mask, dtype=np.float32).reshape(S, S)
    wq = np.asarray(wq, dtype=np.float32)
    wk = np.asarray(wk, dtype=np.float32)
    wv = np.asarray(wv, dtype=np.float32)
    wo = np.asarray(wo, dtype=np.float32)

    if not mask2d.any():
        mode = _MODE_NONE
    elif np.array_equal(mask2d, np.triu(np.ones((S, S), np.float32), k=1)):
        mode = _MODE_CAUSAL
    else:
        mode = _MODE_GENERIC

    if mode not in _NC_CACHE:
        _NC_CACHE[mode] = _build_nc(mode)
    nc = _NC_CACHE[mode]

    in_maps = []
    for core in range(N_CORES):
        b = core // 4
        h0 = (core % 4) * HEADS_PER_CORE
        cs, ce = h0 * DK, (h0 + HEADS_PER_CORE) * DK
        m = {
            "x": np.ascontiguousarray(query_input[b]),
            "wq": np.ascontiguousarray(wq[:, cs:ce]),
            "wk": np.ascontiguousarray(wk[:, cs:ce]),
            "wv": np.ascontiguousarray(wv[:, cs:ce]),
            "wo": np.ascontiguousarray(wo[cs:ce, :]),
        }
        if mode == _MODE_GENERIC:
            m["mexp"] = np.ascontiguousarray(
                np.exp(np.float32(-1e9) * mask2d.T).astype(np.float32))
        in_maps.append(m)

    res = run_bass_kernel_spmd(nc, in_maps, core_ids=list(range(N_CORES)),
                               trace=bool(_profile))
    out = np.zeros((B, S, D), np.float32)
    for core in range(N_CORES):
        out[core // 4] += res.results[core]["out"]
    if _profile:
        return out, res
    return out


# revision 6
# speedup vs baseline: 1.3194x; 1.3194x over previous
"""Multi-head attention (B=2, S=2048, D=1024, H=16) on 8 TRN2 NeuronCores.

Sharding: data-parallel over batch (cores 0-3 -> batch 0, cores 4-7 -> batch 1)
and tensor-parallel over heads (4 heads per core, via column shards of
wq/wk/wv and row shards of wo). Each core computes a partial output
projection [S, D]; the host sums the 4 partials per batch (the "all-reduce"
after the output projection).

Per-core pipeline:
  x -> PE-transpose -> xT (fp32r);  head-pair QT/KT/VT projections (fp32r);
  V via PE transpose with appended ones columns (bf16);  scores for the two
  heads of a pair issued back-to-back into PE row-groups 0-63/64-127
  (concurrent systolic sub-arrays);  causal dead tiles skipped, diagonal
  strips masked multiplicatively post-exp;  softmax without max-subtraction
  (exp row sums come free from the ones column of V);  P in bf16;  O^T
  accumulated in PSUM;  normalization via reciprocal + DMA broadcast;
  partial out = O^T.T @ wo (fp32r).
"""

import numpy as np

import concourse.bass as bass
import concourse.tile as tile
from concourse import bacc, mybir
from concourse.bass_utils import run_bass_kernel_spmd
from concourse.masks import make_identity, make_upper_triangular

B, S, D, H = 2, 2048, 1024, 16
DK = D // H          # 64
N_CORES = 8
HEADS_PER_CORE = H // 4      # 4
NPAIR = HEADS_PER_CORE // 2  # 2 head pairs per core
ND = D // 128        # 8 d-tiles
NS = S // 128        # 16 s-tiles / k-tiles
QW = 512             # q chunk width in the attention loop
NQP = S // QW        # 4 q-chunks

P_BF16 = True        # P (exp output) and V_ext in bf16; False -> fp32r

f32 = mybir.dt.float32
f32r = mybir.dt.float32r
bf16 = mybir.dt.bfloat16

_MODE_NONE, _MODE_CAUSAL, _MODE_GENERIC = 0, 1, 2


def _build_nc(mask_mode: int):
    pdt = bf16 if P_BF16 else f32r
    nc = bacc.Bacc("TRN2", target_bir_lowering=False, debug=False,
                   num_devices=N_CORES)
    x_d = nc.dram_tensor("x", [S, D], f32, kind="ExternalInput").ap()
    wq_d = nc.dram_tensor("wq", [D, 256], f32, kind="ExternalInput").ap()
    wk_d = nc.dram_tensor("wk", [D, 256], f32, kind="ExternalInput").ap()
    wv_d = nc.dram_tensor("wv", [D, 256], f32, kind="ExternalInput").ap()
    wo_d = nc.dram_tensor("wo", [256, D], f32, kind="ExternalInput").ap()
    if mask_mode == _MODE_GENERIC:
        # multiplicative mask, already transposed: mexp[k, q] = exp(-1e9*mask[q, k])
        mexp_d = nc.dram_tensor("mexp", [S, S], f32, kind="ExternalInput").ap()
    out_d = nc.dram_tensor("out", [S, D], f32, kind="ExternalOutput").ap()
    sums_d = nc.dram_tensor("sums_scratch", [NPAIR * NQP * 1024], f32).ap()
    rcp_d = nc.dram_tensor("rcp_scratch", [NPAIR * NQP * 1024], f32).ap()

    with tile.TileContext(nc) as tc:
        with (
            tc.tile_pool(name="const", bufs=1) as const,
            tc.tile_pool(name="big", bufs=1) as big,
            tc.tile_pool(name="pair", bufs=1) as pairp,
            tc.tile_pool(name="vext", bufs=1) as vextp,
            tc.tile_pool(name="ptp", bufs=3) as ptp,
            tc.tile_pool(name="work", bufs=3) as work,
            tc.tile_pool(name="norm", bufs=2) as normp,
            tc.tile_pool(name="psA", bufs=2, space="PSUM") as psA,
            tc.tile_pool(name="psOT", bufs=2, space="PSUM") as psOT,
        ):
            def cp(use_scalar, out, in_):
                if use_scalar:
                    nc.scalar.copy(out, in_)
                else:
                    nc.vector.tensor_copy(out, in_)

            # ---------------- constants (emitted first: gate the transposes)
            ident = const.tile([128, 128], f32)
            make_identity(nc, ident[:])
            ident_r = const.tile([128, 128], f32r)
            nc.vector.tensor_copy(ident_r[:], ident[:])
            tri_f = work.tile([128, 128], f32, name="tri_f", tag="xstage")
            make_upper_triangular(nc, tri_f[:], val=1.0, diag=True)
            tri_m = const.tile([128, 128], pdt)
            nc.vector.tensor_copy(tri_m[:], tri_f[:])
            ones_col = const.tile([128, 1], f32)
            nc.vector.memset(ones_col[:], 1.0)

            # ---------------- x load + transpose -> xT fp32r [128, ND, S]
            xT = big.tile([128, ND, S], f32r)
            for st in range(NS):
                x_t = work.tile([128, D], f32, name="x_t", tag="xstage")
                nc.sync.dma_start(x_t[:], x_d[st * 128:(st + 1) * 128, :])
                tp = psA.tile([128, 1024], f32, name="tp", tag="psA")
                for dt_i in range(ND):
                    nc.tensor.transpose(
                        tp[:, dt_i * 128:(dt_i + 1) * 128],
                        x_t[:, dt_i * 128:(dt_i + 1) * 128], ident[:])
                cp(st % 2 == 0, xT[:, :, st * 128:(st + 1) * 128],
                   tp[:].rearrange("p (d s) -> p d s", d=ND))

            # ---------------- weights: batched loads + round to fp32r
            w_r = {}
            for name, dram in (("wq", wq_d), ("wk", wk_d), ("wv", wv_d)):
                wr = big.tile([128, ND, 256], f32r, name=f"{name}_r",
                              tag=f"{name}_r")
                for hf in range(2):
                    wf = work.tile([128, 4, 256], f32, name="wf", tag="xstage")
                    nc.sync.dma_start(
                        wf[:], dram[hf * 512:(hf + 1) * 512, :].rearrange(
                            "(t p) m -> p t m", p=128))
                    cp(hf == 0, wr[:, hf * 4:(hf + 1) * 4, :], wf[:])
                w_r[name] = wr
            wo_r = big.tile([128, 2, D], f32r)
            for t2 in range(2):
                wf2 = work.tile([128, D], f32, name="wf2", tag="xstage")
                nc.sync.dma_start(wf2[:], wo_d[t2 * 128:(t2 + 1) * 128, :])
                cp(t2 == 0, wo_r[:, t2, :], wf2[:])

            ot_all = big.tile([128, NPAIR, S], f32r)  # normalized O^T, head pairs

            for pair in range(NPAIR):
                # ------------ projections for this head pair (2 heads packed)
                co = pair * 128
                proj = {}
                for name in ("wq", "wk", "wv"):
                    pr = pairp.tile([128, S], f32r, name=f"proj_{name}",
                                    tag=f"proj_{name}")
                    for qc4 in range(4):   # 4 chunks of 512
                        pj = psA.tile([128, 1024], f32, name="pj", tag="psA")
                        for dt_i in range(ND):
                            nc.tensor.matmul(
                                pj[:, 0:512],
                                w_r[name][:, dt_i, co:co + 128],
                                xT[:, dt_i, qc4 * 512:(qc4 + 1) * 512],
                                start=(dt_i == 0), stop=(dt_i == ND - 1))
                        cp(qc4 % 2 == 0, pr[:, qc4 * 512:(qc4 + 1) * 512],
                           pj[:, 0:512])
                    proj[name] = pr

                # ------------ V_ext: [VA(0:64)|onesA(64)|VB(65:129)|onesB(129)]
                v_ext = vextp.tile([128, NS, 130], pdt, name="v_ext",
                                   tag="v_ext")
                for kt in range(NS):
                    vp = psA.tile([128, 1024], f32r, name="vp", tag="psA")
                    nc.tensor.transpose(vp[:, 0:128],
                                        proj["wv"][:, kt * 128:(kt + 1) * 128],
                                        ident_r[:])
                    cp(False, v_ext[:, kt, 0:64], vp[:, 0:64])
                    cp(False, v_ext[:, kt, 65:129], vp[:, 64:128])
                    nc.vector.tensor_copy(v_ext[:, kt, 64:65], ones_col[:])
                    nc.vector.tensor_copy(v_ext[:, kt, 129:130], ones_col[:])

                qtA = proj["wq"][0:64, :]
                qtB = proj["wq"][64:128, :]
                ktA = proj["wk"][0:64, :]
                ktB = proj["wk"][64:128, :]
                for qp in range(NQP):    # q-chunks of 512, both heads packed
                    ot = psOT.tile([65, 1024], f32, name="ot", tag="ot")
                    kt_end = 4 * (qp + 1) if mask_mode == _MODE_CAUSAL else NS
                    for kt in range(kt_end):
                        lo = (max(0, kt * 128 - qp * QW)
                              if mask_mode == _MODE_CAUSAL else 0)
                        ks = slice(kt * 128, (kt + 1) * 128)
                        qs = slice(qp * QW + lo, (qp + 1) * QW)
                        st_ps = psA.tile([128, 1024], f32, name="st_ps",
                                         tag="psA")
                        # two heads back-to-back -> concurrent PE row groups
                        nc.tensor.matmul(st_ps[:, lo:512], ktA[:, ks],
                                         qtA[:, qs], start=True, stop=True)
                        nc.tensor.matmul(st_ps[:, 512 + lo:1024], ktB[:, ks],
                                         qtB[:, qs], start=True, stop=True)
                        pt = ptp.tile([128, 1024], pdt, name="pt", tag="pt")
                        if lo == 0:
                            nc.scalar.activation(
                                pt[:], st_ps[:],
                                mybir.ActivationFunctionType.Exp,
                                scale=1.0 / float(np.sqrt(DK)))
                        else:
                            # one op spanning both heads' live columns; the
                            # unwritten PSUM gap [512, 512+lo) is exp'd into
                            # pt columns that no O matmul reads
                            nc.scalar.activation(
                                pt[:, lo:1024], st_ps[:, lo:1024],
                                mybir.ActivationFunctionType.Exp,
                                scale=1.0 / float(np.sqrt(DK)))
                        if mask_mode == _MODE_CAUSAL and kt * 128 >= qp * QW:
                            nc.vector.tensor_mul(pt[:, lo:lo + 128],
                                                 pt[:, lo:lo + 128], tri_m[:])
                            nc.vector.tensor_mul(
                                pt[:, 512 + lo:512 + lo + 128],
                                pt[:, 512 + lo:512 + lo + 128], tri_m[:])
                        elif mask_mode == _MODE_GENERIC:
                            mm_t = work.tile([128, QW], f32, name="mm_t",
                                             tag="mm_t")
                            nc.sync.dma_start(
                                mm_t[:], mexp_d[ks, qp * QW:(qp + 1) * QW])
                            nc.vector.tensor_mul(pt[:, 0:512], pt[:, 0:512],
                                                 mm_t[:])
                            nc.vector.tensor_mul(pt[:, 512:1024],
                                                 pt[:, 512:1024], mm_t[:])
                        nc.tensor.matmul(ot[:, lo:512],
                                         v_ext[:, kt, 0:65], pt[:, lo:512],
                                         start=(kt == 0),
                                         stop=(kt == kt_end - 1))
                        nc.tensor.matmul(ot[:, 512 + lo:1024],
                                         v_ext[:, kt, 65:130],
                                         pt[:, 512 + lo:1024],
                                         start=(kt == 0),
                                         stop=(kt == kt_end - 1))
                    # ------ normalize: one chain per (pair, qp), both heads
                    off = (pair * NQP + qp) * 1024
                    sums_sb = normp.tile([1, 1024], f32, name="sums_sb",
                                         tag="sums_sb")
                    cp(False, sums_sb[:], ot[64:65, :])
                    nc.sync.dma_start(sums_d[None, off:off + 1024],
                                      sums_sb[0:1, :])
                    rsr = normp.tile([128, 16], f32, name="rsr", tag="rsr")
                    nc.sync.dma_start(
                        rsr[:, 0:8], sums_d[off:off + 1024].rearrange(
                            "(p f) -> p f", f=8))
                    nc.vector.reciprocal(out=rsr[:, 8:16], in_=rsr[:, 0:8])
                    nc.sync.dma_start(
                        rcp_d[off:off + 1024].rearrange("(p f) -> p f", f=8),
                        rsr[:, 8:16])
                    rb = normp.tile([64, 1024], f32, name="rb", tag="rb")
                    nc.sync.dma_start(
                        rb[:], rcp_d[None, off:off + 1024].to_broadcast(
                            (64, 1024)))
                    nc.vector.tensor_mul(
                        ot_all[0:64, pair, qp * QW:(qp + 1) * QW],
                        ot[0:64, 0:512], rb[:, 0:512])
                    nc.vector.tensor_mul(
                        ot_all[64:128, pair, qp * QW:(qp + 1) * QW],
                        ot[0:64, 512:1024], rb[:, 512:1024])

            # ---------------- output projection (partial): out = OT_all.T @ wo
            for qt in range(NS):
                op = psA.tile([128, 1024], f32, name="op", tag="psA")
                for half in range(2):
                    for pair in range(NPAIR):
                        nc.tensor.matmul(
                            op[:, half * 512:(half + 1) * 512],
                            ot_all[:, pair, qt * 128:(qt + 1) * 128],
                            wo_r[:, pair, half * 512:(half + 1) * 512],
                            start=(pair == 0), stop=(pair == NPAIR - 1))
                o_sb = work.tile([128, D], f32, name="o_sb", tag="xstage")
                cp(qt % 2 == 0, o_sb[:], op[:])
                nc.sync.dma_start(out_d[qt * 128:(qt + 1) * 128, :], o_sb[:])

    nc.compile()
    return nc


_NC_CACHE = {}


def kernel(query_input, mask, wq, wk, wv, wo, _profile=False):
    query_input = np.asarray(query_input, dtype=np.float32)
    mask2d = np.asarray(mask, dtype=np.float32).reshape(S, S)
    wq = np.asarray(wq, dtype=np.float32)
    wk = np.asarray(wk, dtype=np.float32)
    wv = np.asarray(wv, dtype=np.float32)
    wo = np.asarray(wo, dtype=np.float32)

    if not mask2d.any():
        mode = _MODE_NONE
    elif np.array_equal(mask2d, np.triu(np.ones((S, S), np.float32), k=1)):
        mode = _MODE_CAUSAL
    else:
        mode = _MODE_GENERIC

    if mode not in _NC_CACHE:
        _NC_CACHE[mode] = _build_nc(mode)
    nc = _NC_CACHE[mode]

    in_maps = []
    for core in range(N_CORES):
        b = core // 4
        h0 = (core % 4) * HEADS_PER_CORE
        cs, ce = h0 * DK, (h0 + HEADS_PER_CORE) * DK
        m = {
            "x": np.ascontiguousarray(query_input[b]),
            "wq": np.ascontiguousarray(wq[:, cs:ce]),
            "wk": np.ascontiguousarray(wk[:, cs:ce]),
            "wv": np.ascontiguousarray(wv[:, cs:ce]),
            "wo": np.ascontiguousarray(wo[cs:ce, :]),
        }
        if mode == _MODE_GENERIC:
            m["mexp"] = np.ascontiguousarray(
                np.exp(np.float32(-1e9) * mask2d.T).astype(np.float32))
        in_maps.append(m)

    res = run_bass_kernel_spmd(nc, in_maps, core_ids=list(range(N_CORES)),
                               trace=bool(_profile))
    out = np.zeros((B, S, D), np.float32)
    for core in range(N_CORES):
        out[core // 4] += res.results[core]["out"]
    if _profile:
        return out, res
    return out
